# revision 1
# baseline (speedup 1.0000x reference)
"""Trainium2 Bass kernel for ColorImageLoss (gaussian-blur + bilinear grid
sample + MSE), data-parallel over batch across 8 NeuronCores.

Key idea: the loss only reads the blurred image at 64 sample points per
image.  Each bilinear sample needs a 2x2 patch of blurred pixels; the 7-tap
separable blur support of those pixels is an 8x8 patch of the *original*
image.  Reflect padding folds into per-sample 8-tap row/col weight vectors
(reflected tap indices provably stay inside the clamped 8-wide window
[clamp(x0-3,0,W-8), +8)).  So per sample we indirect-DMA-gather an 8x8x3
patch and compute  target_c = vw^T @ Patch_c @ hw  on device, then the MSE
partial sum.  HBM traffic: ~200KB/core instead of 12.6MB/core.
"""

import os
import sys

import numpy as np

for _p in ("/opt/trn_rl_repo", "/root/.axon_site/_ro/trn_rl_repo"):
    if os.path.isdir(_p) and _p not in sys.path:
        sys.path.insert(0, _p)

import concourse.bass as bass
import concourse.mybir as mybir
import concourse.tile as tile
from concourse.bass_utils import run_bass_kernel_spmd

# Problem geometry (hardcoded per contract)
B, L, NCH, H, W = 32, 64, 3, 512, 512
NCORES = 8
BPC = B // NCORES            # images per core
NS = BPC * L                 # samples per core (256)
P = 128                      # SBUF partitions
SLOTS = NS // P              # 2 sample slots per partition
KS = 7                       # blur taps
IMG_ELEMS = BPC * NCH * H * W

f32 = mybir.dt.float32
i32 = mybir.dt.int32
Alu = mybir.AluOpType
Ax = mybir.AxisListType

# meta tensor per-partition layout (f32 columns)
O_POS = 0            # [SLOTS, 2] (x, y)                -> 4
O_COL = 4            # [SLOTS, 3] color                 -> 6
O_JM3 = 10           # [4, 7] j-3 replicated per var    -> 28
O_KK = 38            # [7] blur kernel                  -> 7
O_IB = 45            # [SLOTS, 3, 8] gather index base  -> 48
O_IOTA8 = 93         # [8] 0..7                         -> 8
META_W = 101


def _gauss_kernel_np():
    x = (np.arange(KS, dtype=np.float32) - (KS - 1) / 2).astype(np.float32)
    k = np.exp(-0.5 * (x / np.float32(1.0)) ** 2).astype(np.float32)
    return (k / k.sum()).astype(np.float32)


def _fap(t, dims, extra_offset=0):
    """AP over tile `t` keeping its partition dim, replacing free dims.

    dims: list of [step, count] in elements; step 0 broadcasts.
    """
    base = t[:] if hasattr(t, "tile") else t
    return bass.AP(
        base.tensor, base.offset + extra_offset,
        [list(base.ap[0])] + [list(d) for d in dims],
    )


def split_multi_waits(nc):
    """This walrus encodes at most ONE sync wait per TPB instruction.  Hoist
    extra waits onto same-engine NoOps inserted directly before the
    instruction (the sequencer executes waits in queue order, so semantics
    are identical)."""
    n_split = 0
    for f in nc.m.functions:
        for blk in f.blocks:
            insts = blk.instructions
            i = 0
            while i < len(insts):
                inst = insts[i]
                si = inst.sync_info
                if si is not None and si.on_wait is not None and len(si.on_wait) > 1:
                    waits = list(si.on_wait)
                    for w in waits[:-1]:
                        nop = mybir.InstNoOp(
                            name=f"{inst.name}-wsplit{n_split}",
                            engine=inst.engine,
                            ins=[],
                            outs=[],
                            sync_info=mybir.SyncInfo(on_wait=[w], on_update=[]),
                        )
                        nc.register_instruction(nop, overwrite=True)
                        insts.insert(i, nop)
                        i += 1
                        n_split += 1
                    inst.sync_info = mybir.SyncInfo(
                        on_wait=[waits[-1]], on_update=list(si.on_update or []))
                i += 1
    return n_split


def build_bass(debug_taps=False, repeat=1, skip_gather=False, skip_compute=False):
    nc = bass.Bass("TRN2")

    img = nc.dram_tensor("img", [IMG_ELEMS, 1], f32, kind="ExternalInput")
    meta = nc.dram_tensor("meta", [P, META_W], f32, kind="ExternalInput")
    partial = nc.dram_tensor("partial", [P, 1], f32, kind="ExternalOutput")

    taps = []

    def tap(name, t, width):
        if not debug_taps:
            return
        d = nc.dram_tensor(f"tap_{name}", [P, width], f32, kind="ExternalOutput")
        taps.append((name, t, d, width))

    with tile.TileContext(nc) as tc:
        for _rep in range(repeat):
            with tc.tile_pool(name=f"main{_rep}", bufs=1) as pool:
                m = pool.tile([P, META_W], f32)
                nc.sync.dma_start(out=m[:], in_=meta[:])

                # ---- positions -> x,y (reference op order, f32) ----
                # gx = 2p-1 ; v = ((gx+1)*W - 1)*0.5 ; clip [0, W-1]
                xy = pool.tile([P, SLOTS, 2], f32)
                pos_ap = _fap(m, [[2, SLOTS], [1, 2]], O_POS)
                nc.vector.tensor_scalar(xy[:], pos_ap, 2.0, -1.0, Alu.mult, Alu.add)
                nc.vector.tensor_scalar(xy[:], xy[:], 1.0, float(W), Alu.add, Alu.mult)
                nc.vector.tensor_scalar(xy[:], xy[:], -1.0, 0.5, Alu.add, Alu.mult)
                nc.vector.tensor_scalar(xy[:], xy[:], 0.0, float(W - 1), Alu.max, Alu.min)
                tap('xy', xy, 4)

                # ---- floor/frac ----
                # floor via the exact round-to-nearest trick: (x + 2^23) - 2^23
                # rounds to integer (f32 grid at 2^23 is 1.0); subtract the
                # rounded-up-past-x case.  Two separate instructions so each
                # result rounds to f32 in SBUF.
                rnd = pool.tile([P, SLOTS, 2], f32)
                nc.vector.tensor_scalar(rnd[:], xy[:], 8388608.0, None, Alu.add)
                nc.vector.tensor_scalar(rnd[:], rnd[:], -8388608.0, None, Alu.add)
                gtx = pool.tile([P, SLOTS, 2], f32)
                nc.vector.tensor_tensor(gtx[:], rnd[:], xy[:], op=Alu.is_gt)
                wxy = pool.tile([P, SLOTS, 2], f32)   # frac (wx, wy)
                fxy = pool.tile([P, SLOTS, 2], f32)   # floor (x0, y0)
                nc.vector.tensor_sub(fxy[:], rnd[:], gtx[:])
                nc.vector.tensor_sub(wxy[:], xy[:], fxy[:])
                tap('fxy', fxy, 4)
                tap('wxy', wxy, 4)

                # ---- p4 [P, SLOTS, 2axis, 2which]: (x0, x1, y0, y1) ----
                p4 = pool.tile([P, SLOTS, 2, 2], f32)
                nc.vector.tensor_copy(_fap(p4, [[4, SLOTS], [2, 2], [1, 1]]), fxy[:])
                nc.vector.tensor_scalar(
                    _fap(p4, [[4, SLOTS], [2, 2], [1, 1]], 1),
                    fxy[:], 1.0, float(W - 1), Alu.add, Alu.min)
                tap('p4', p4, 8)

                # ---- window starts s = clamp(floor-3, 0, W-8) [P, SLOTS, 2] ----
                s_t = pool.tile([P, SLOTS, 2], f32)
                nc.vector.tensor_scalar(s_t[:], fxy[:], -3.0, 0.0, Alu.add, Alu.max)
                nc.vector.tensor_scalar(s_t[:], s_t[:], float(W - 8), None, Alu.min)
                tap('s_t', s_t, 4)

                # ---- gather indices [P, SLOTS, 3ch, 8row] ----
                rc = pool.tile([P, SLOTS], f32)       # sy*W + sx
                nc.vector.tensor_scalar(
                    rc[:], _fap(s_t, [[2, SLOTS], [1, 1]], 1), float(W), None, Alu.mult)
                nc.vector.tensor_tensor(
                    rc[:], rc[:], _fap(s_t, [[2, SLOTS], [1, 1]]), op=Alu.add)
                idxf = pool.tile([P, SLOTS, NCH, 8], f32)
                ib = _fap(m, [[24, SLOTS], [8, NCH], [1, 8]], O_IB)
                rc_b = _fap(rc, [[1, SLOTS], [0, NCH * 8]])
                nc.vector.tensor_tensor(
                    _fap(idxf, [[NCH * 8, SLOTS], [1, NCH * 8]]), ib, rc_b, op=Alu.add)
                tap('idxf', idxf, 48)
                idx = pool.tile([P, SLOTS, NCH, 8], i32)
                nc.vector.tensor_copy(idx[:], idxf[:])

                # ---- indirect gather: 8 contiguous pixels per index ----
                # HW SWDGE pairs ONE index per partition-row descriptor per call
                # (probe-verified; multi-index-per-partition layouts misbehave).
                # So issue one call per segment column: each call gathers one
                # 8-px run per partition using a [P, 1] index slice.
                patches = pool.tile([P, SLOTS, NCH, 8, 8], f32)
                for seg in range(0 if skip_gather else SLOTS * NCH * 8):
                    nc.gpsimd.indirect_dma_start(
                        out=_fap(patches, [[1, 8]], 8 * seg),
                        out_offset=None,
                        in_=img[:],
                        in_offset=bass.IndirectOffsetOnAxis(
                            ap=_fap(idx, [[1, 1]], seg), axis=0),
                    )


                # ---- raw tap positions T [P, SLOTS, 4var, 7] = p4 + (j-3) ----
                t_t = pool.tile([P, SLOTS, 4, KS], f32)
                p4_b = _fap(p4, [[4, SLOTS], [1, 4], [0, KS]])
                jm3 = _fap(m, [[0, SLOTS], [KS, 4], [1, KS]], O_JM3)
                nc.vector.tensor_add(t_t[:], p4_b, jm3)
                tap('t_t', t_t, 56)

                # ---- reflect: R = min(abs(T), 2*(W-1) - T); abs = max(T, -T) ----
                neg = pool.tile([P, SLOTS, 4, KS], f32)
                a_t = pool.tile([P, SLOTS, 4, KS], f32)
                b_t = pool.tile([P, SLOTS, 4, KS], f32)
                nc.vector.tensor_scalar(neg[:], t_t[:], -1.0, None, Alu.mult)
                nc.vector.tensor_tensor(a_t[:], t_t[:], neg[:], op=Alu.max)
                nc.vector.tensor_scalar(
                    b_t[:], t_t[:], -1.0, float(2 * (W - 1)), Alu.mult, Alu.add)
                r_t = pool.tile([P, SLOTS, 4, KS], f32)
                nc.vector.tensor_tensor(r_t[:], a_t[:], b_t[:], op=Alu.min)
                tap('r_t', r_t, 56)

                # ---- window-relative tap Z = R - s(axis)  in [0,8) ----
                z_t = pool.tile([P, SLOTS, 4, KS], f32)
                s_b = _fap(s_t, [[2, SLOTS], [1, 2], [0, 2 * KS]])
                r_v = _fap(r_t, [[4 * KS, SLOTS], [2 * KS, 2], [1, 2 * KS]])
                z_v = _fap(z_t, [[4 * KS, SLOTS], [2 * KS, 2], [1, 2 * KS]])
                nc.vector.tensor_tensor(z_v, r_v, s_b, op=Alu.subtract)
                tap('z_t', z_t, 56)

                # ---- per-window-offset kernel weights K [P, SLOTS, 4var, 8] ----
                # K[v, u] = sum_j kk[j] * (Z[v, j] == u)
                eq = pool.tile([P, SLOTS * 4, 8, KS], f32)
                z_b = _fap(z_t, [[KS, SLOTS * 4], [0, 8], [1, KS]])
                iota_b = _fap(m, [[0, SLOTS * 4], [1, 8], [0, KS]], O_IOTA8)
                nc.vector.tensor_tensor(eq[:], z_b, iota_b, op=Alu.is_equal)
                kk_b = _fap(m, [[0, SLOTS * 4], [0, 8], [1, KS]], O_KK)
                nc.vector.tensor_tensor(eq[:], eq[:], kk_b, op=Alu.mult)
                tap('eqk', eq, 448)
                kw = pool.tile([P, SLOTS, 4, 8], f32)
                nc.vector.tensor_reduce(
                    out=_fap(kw, [[1, SLOTS * 4 * 8]]),
                    in_=eq[:], axis=Ax.X, op=Alu.add)
                tap('kw', kw, 64)

                # ---- bilinear weights ww [P, SLOTS, 2axis, 2which] ----
                ww = pool.tile([P, SLOTS, 2, 2], f32)
                nc.vector.tensor_copy(_fap(ww, [[4, SLOTS], [2, 2], [1, 1]], 1), wxy[:])
                nc.vector.tensor_scalar(
                    _fap(ww, [[4, SLOTS], [2, 2], [1, 1]]),
                    wxy[:], -1.0, 1.0, Alu.mult, Alu.add)

                # ---- vh = K * ww  -> summed over which -> axis taps [P,SLOTS,2,8]
                vh = pool.tile([P, SLOTS, 4, 8], f32)
                ww_b = _fap(ww, [[4, SLOTS], [1, 4], [0, 8]])
                nc.vector.tensor_tensor(vh[:], kw[:], ww_b, op=Alu.mult)
                hwv = pool.tile([P, SLOTS, 2, 8], f32)   # axis 0 = x taps, 1 = y taps
                vh0 = _fap(vh, [[32, SLOTS], [16, 2], [1, 8]])
                vh1 = _fap(vh, [[32, SLOTS], [16, 2], [1, 8]], 8)
                nc.vector.tensor_tensor(hwv[:], vh0, vh1, op=Alu.add)
                tap('hwv', hwv, 32)

                # ---- outer product wp[u,t] = vw[u]*hw[t] [P, SLOTS, 8, 8] ----
                wp = pool.tile([P, SLOTS, 8, 8], f32)
                vw_b = _fap(hwv, [[16, SLOTS], [1, 8], [0, 8]], 8)   # y taps (rows)
                hw_b = _fap(hwv, [[16, SLOTS], [0, 8], [1, 8]])      # x taps (cols)
                nc.vector.tensor_tensor(wp[:], vw_b, hw_b, op=Alu.mult)
                tap('wp', wp, 128)

                # ---- apply weights, reduce to target, MSE partial ----
                # Wait-splitter: compute instructions encode at most one sync
                # wait.  This copy's only dependency is the gather DMA, so it
                # absorbs the DMASW wait; the multiply below then only needs the
                # same-engine DVE chain wait.
                dummy = pool.tile([P, 1], f32)
                nc.vector.tensor_copy(dummy[:], _fap(patches, [[1, 1]]))
                tap('patches', patches, 384)
                tmp = pool.tile([P, SLOTS, NCH, 64], f32)
                wp_b = _fap(wp, [[64, SLOTS], [0, NCH], [1, 64]])
                pat_v = _fap(patches, [[NCH * 64, SLOTS], [64, NCH], [1, 64]])
                nc.vector.tensor_tensor(tmp[:], pat_v, wp_b, op=Alu.mult)
                tap('tmp', tmp, 384)
                tgt = pool.tile([P, SLOTS, NCH], f32)
                nc.vector.tensor_reduce(
                    out=_fap(tgt, [[1, SLOTS * NCH]]),
                    in_=_fap(tmp, [[64, SLOTS * NCH], [1, 64]]),
                    axis=Ax.X, op=Alu.add)
                tap('tgt', tgt, 6)
                diff = pool.tile([P, SLOTS, NCH], f32)
                col_ap = _fap(m, [[NCH, SLOTS], [1, NCH]], O_COL)
                nc.vector.tensor_tensor(diff[:], tgt[:], col_ap, op=Alu.subtract)
                sq = pool.tile([P, SLOTS, NCH], f32)
                nc.vector.tensor_tensor(sq[:], diff[:], diff[:], op=Alu.mult)
                part = pool.tile([P, 1], f32)
                nc.vector.tensor_reduce(
                    out=part[:], in_=_fap(sq, [[1, SLOTS * NCH]]), axis=Ax.X, op=Alu.add)

                nc.sync.dma_start(out=partial[:], in_=part[:])

                for _name, _t, _d, _w in taps:
                    nc.sync.dma_start(out=_d[:], in_=_fap(_t, [[1, _w]]))

    split_multi_waits(nc)
    return nc


def make_meta(pred_shard):
    """Build the per-core [P, META_W] meta tensor from the [BPC, L, 8]
    predictions shard.  Sample i = slot*P + p."""
    flat = np.ascontiguousarray(pred_shard.reshape(NS, 8).astype(np.float32))
    meta = np.zeros((P, META_W), dtype=np.float32)
    pos = flat[:, :2].reshape(SLOTS, P, 2).transpose(1, 0, 2)     # [P,SLOTS,2]
    col = flat[:, 5:8].reshape(SLOTS, P, 3).transpose(1, 0, 2)    # [P,SLOTS,3]
    meta[:, O_POS:O_POS + 4] = pos.reshape(P, 4)
    meta[:, O_COL:O_COL + 6] = col.reshape(P, 6)
    jm3 = np.tile((np.arange(KS, dtype=np.float32) - 3.0), 4)     # [4*7]
    meta[:, O_JM3:O_JM3 + 28] = jm3[None, :]
    meta[:, O_KK:O_KK + KS] = _gauss_kernel_np()[None, :]
    # gather index base: img(slot,p) * CH*H*W + c*H*W + u*W
    p_idx = np.arange(P)
    base = np.zeros((P, SLOTS, NCH, 8), dtype=np.float32)
    for slot in range(SLOTS):
        img_i = (slot * P + p_idx) // L                           # [P]
        for c in range(NCH):
            for u in range(8):
                base[:, slot, c, u] = (
                    img_i * (NCH * H * W) + c * (H * W) + u * W)
    meta[:, O_IB:O_IB + 48] = base.reshape(P, 48)
    meta[:, O_IOTA8:O_IOTA8 + 8] = np.arange(8, dtype=np.float32)[None, :]
    return meta


def make_in_maps(predictions, ref_imgs):
    in_maps = []
    for k in range(NCORES):
        img_shard = np.ascontiguousarray(
            ref_imgs[k * BPC:(k + 1) * BPC].astype(np.float32)).reshape(-1, 1)
        meta = make_meta(predictions[k * BPC:(k + 1) * BPC])
        in_maps.append({"img": img_shard, "meta": meta})
    return in_maps


_NC_CACHE = {}


def get_nc():
    if "nc" not in _NC_CACHE:
        _NC_CACHE["nc"] = build_bass()
    return _NC_CACHE["nc"]


def _reduce_results(res):
    total = np.float64(0.0)
    for r in res.results:
        total += np.float64(r["partial"].sum(dtype=np.float64))
    return np.float32(total / (B * L * NCH))


def kernel(predictions, ref_imgs):
    predictions = np.asarray(predictions)
    ref_imgs = np.asarray(ref_imgs)
    nc = get_nc()
    in_maps = make_in_maps(predictions, ref_imgs)
    res = run_bass_kernel_spmd(nc, in_maps, list(range(NCORES)))
    return _reduce_results(res)


def run_profiled(predictions, ref_imgs):
    """Like kernel(), but traces with neuron-profile; returns (loss, results)."""
    predictions = np.asarray(predictions)
    ref_imgs = np.asarray(ref_imgs)
    nc = get_nc()
    in_maps = make_in_maps(predictions, ref_imgs)
    res = run_bass_kernel_spmd(
        nc, in_maps, list(range(NCORES)), trace=True)
    return _reduce_results(res), res



# revision 4
# speedup vs baseline: 3.0413x; 3.0413x over previous
"""Trainium2 Bass kernel for ColorImageLoss (gaussian-blur + bilinear grid
sample + MSE), data-parallel over batch across 8 NeuronCores.

The loss only reads the blurred image at 64 sample points per image; each
sample's support in the ORIGINAL image is an 8(cols) x 8(rows) x 3(ch) patch
(7-tap blur + bilinear, reflect padding folds into per-sample 8-tap weights).

v2 gather strategy: HW SWDGE generates ONE descriptor per partition per
indirect-DMA call (one index, contiguous run) with ~1us fixed cost per CALL,
so the baseline's 48 runs/partition => 48 calls => ~50us of serialized
descriptor generation.  Fix: re-layout the image on the host into overlapping
16-row "bands" [b, band, x, row, ch] so that any sample's whole support is
ONE contiguous 1536B run (8 cols x 16 rows x 3 ch); 2 samples/partition =>
2 indirect calls total.  Row weights become 16-slot (band-relative), column
weights stay 8-slot; target = vw16^T Patch hw8 per channel, then MSE.
"""

import os
import sys

import numpy as np
from numpy.lib.stride_tricks import as_strided

for _p in ("/opt/trn_rl_repo", "/root/.axon_site/_ro/trn_rl_repo"):
    if os.path.isdir(_p) and _p not in sys.path:
        sys.path.insert(0, _p)

import concourse.bass as bass
import concourse.mybir as mybir
import concourse.tile as tile
from concourse.bass_utils import run_bass_kernel_spmd

# Problem geometry (hardcoded per contract)
B, L, NCH, H, W = 32, 64, 3, 512, 512
NCORES = 8
BPC = B // NCORES            # images per core
NS = BPC * L                 # samples per core (256)
P = 128                      # SBUF partitions
SLOTS = NS // P              # 2 sample slots per partition
KS = 7                       # blur taps

NB = 63                      # bands per image (starts 0,8,...,496)
BH = 16                      # band height (rows)
XSTRIDE = BH * NCH           # 48   elems per band column
BSTRIDE = W * XSTRIDE        # 24576 elems per band
IMG_BANDS = NB * BSTRIDE     # elems per image in band layout
BANDS_ELEMS = BPC * IMG_BANDS
RUN = 8 * XSTRIDE            # 384 elems gathered per sample

f32 = mybir.dt.float32
i32 = mybir.dt.int32
Alu = mybir.AluOpType
Ax = mybir.AxisListType

# meta tensor per-partition layout (f32 columns)
O_POS = 0            # [SLOTS, 2] (x, y)       -> 4
O_COL = 4            # [SLOTS, 3] color        -> 6
O_JM3 = 10           # [7] j-3                 -> 7
O_KK = 17            # [7] blur kernel         -> 7
O_IB = 24            # [SLOTS] img band base   -> 2
O_IOTA = 26          # [16] 0..15              -> 16
META_W = 42

MAGIC = 12582912.0   # 1.5 * 2^23: add/sub rounds f32 in [0, 2^22) to nearest int


def _gauss_kernel_np():
    x = (np.arange(KS, dtype=np.float32) - (KS - 1) / 2).astype(np.float32)
    k = np.exp(-0.5 * (x / np.float32(1.0)) ** 2).astype(np.float32)
    return (k / k.sum()).astype(np.float32)


def _fap(t, dims, extra_offset=0):
    """AP over tile `t` keeping its partition dim, replacing free dims.

    dims: list of [step, count] in elements; step 0 broadcasts.
    """
    base = t[:] if hasattr(t, "tile") else t
    return bass.AP(
        base.tensor, base.offset + extra_offset,
        [list(base.ap[0])] + [list(d) for d in dims],
    )


def split_multi_waits(nc):
    """This walrus encodes at most ONE sync wait per TPB instruction.  Hoist
    extra waits onto same-engine NoOps inserted directly before the
    instruction (the sequencer executes waits in queue order, so semantics
    are identical)."""
    n_split = 0
    for f in nc.m.functions:
        for blk in f.blocks:
            insts = blk.instructions
            i = 0
            while i < len(insts):
                inst = insts[i]
                si = inst.sync_info
                if si is not None and si.on_wait is not None and len(si.on_wait) > 1:
                    waits = list(si.on_wait)
                    for w in waits[:-1]:
                        nop = mybir.InstNoOp(
                            name=f"{inst.name}-wsplit{n_split}",
                            engine=inst.engine,
                            ins=[],
                            outs=[],
                            sync_info=mybir.SyncInfo(on_wait=[w], on_update=[]),
                        )
                        nc.register_instruction(nop, overwrite=True)
                        insts.insert(i, nop)
                        i += 1
                        n_split += 1
                    inst.sync_info = mybir.SyncInfo(
                        on_wait=[waits[-1]], on_update=list(si.on_update or []))
                i += 1
    return n_split


def build_bass(repeat=1):
    nc = bass.Bass("TRN2")

    bands = nc.dram_tensor("bands", [BANDS_ELEMS, 1], f32, kind="ExternalInput")
    meta = nc.dram_tensor("meta", [P, META_W], f32, kind="ExternalInput")
    partial = nc.dram_tensor("partial", [P, 1], f32, kind="ExternalOutput")

    with tile.TileContext(nc) as tc:
        for _rep in range(repeat):
            with tc.tile_pool(name=f"main{_rep}", bufs=1) as pool:
                m = pool.tile([P, META_W], f32)
                nc.sync.dma_start(out=m[:], in_=meta[:])

                V = nc.vector      # DVE
                G = nc.gpsimd      # Pool

                # ---- positions -> x,y in [0, 511] ----
                # x = clip(512*p - 0.5, 0, 511)  (continuous-equivalent to the
                # reference's op order; differences only shift floor at exact
                # integer boundaries where bilinear weights make it a no-op)
                xy = pool.tile([P, SLOTS, 2], f32)
                pos_ap = _fap(m, [[2, SLOTS], [1, 2]], O_POS)
                V.tensor_scalar(xy[:], pos_ap, 512.0, -0.5, Alu.mult, Alu.add)
                V.tensor_scalar(xy[:], xy[:], 0.0, 511.0, Alu.max, Alu.min)

                # ---- floor via round-to-nearest(x - 0.5) (ties are benign:
                # bilinear interp is continuous across integer boundaries) ----
                fxy = pool.tile([P, SLOTS, 2], f32)
                V.tensor_scalar(fxy[:], xy[:], -0.5, MAGIC, Alu.add, Alu.add)
                V.tensor_scalar(fxy[:], fxy[:], -MAGIC, None, Alu.add)

                # ---- window starts (unclamped-above) ----
                sraw = pool.tile([P, SLOTS, 2], f32)
                V.tensor_scalar(sraw[:], fxy[:], -3.0, 0.0, Alu.add, Alu.max)
                sraw_x = _fap(sraw, [[2, SLOTS], [1, 1]])
                sraw_y = _fap(sraw, [[2, SLOTS], [1, 1]], 1)

                # ---- band index m = clamp(floor(s_y/8), 0, 62) ----
                # floor via round(v - 0.5); a tie (s_y % 8 == 0) may round down
                # to m-1, whose 16-row band still contains the 8-row window.
                vm = pool.tile([P, SLOTS], f32)
                V.tensor_scalar(vm[:], sraw_y, -4.0, 0.125, Alu.add, Alu.mult)
                V.tensor_scalar(vm[:], vm[:], MAGIC, None, Alu.add)
                mhat = pool.tile([P, SLOTS], f32)
                V.tensor_scalar(mhat[:], vm[:], -MAGIC, 62.0, Alu.add, Alu.min)

                # ---- gather index = ib + m*BSTRIDE + min(s_x,504)*XSTRIDE ----
                sxc = pool.tile([P, SLOTS], f32)
                V.tensor_scalar(sxc[:], sraw_x, 504.0, None, Alu.min)
                rc = pool.tile([P, SLOTS], f32)
                V.tensor_scalar(rc[:], sxc[:], float(XSTRIDE), None, Alu.mult)
                mb = pool.tile([P, SLOTS], f32)
                V.tensor_scalar(mb[:], mhat[:], float(BSTRIDE), None, Alu.mult)
                V.tensor_tensor(rc[:], rc[:], mb[:], op=Alu.add)
                ib_ap = _fap(m, [[1, SLOTS]], O_IB)
                V.tensor_tensor(rc[:], rc[:], ib_ap, op=Alu.add)
                idx = pool.tile([P, SLOTS], i32)
                V.tensor_copy(idx[:], rc[:])

                # ---- 2 indirect gathers: one 384-elem run per sample ----
                # patches[p, slot, dx, r, c] = img[img_i, c, 8*m + r, s_x + dx]
                patches = pool.tile([P, SLOTS, 8, BH, NCH], f32)
                for slot in range(SLOTS):
                    G.indirect_dma_start(
                        out=_fap(patches, [[1, RUN]], RUN * slot),
                        out_offset=None,
                        in_=bands[:],
                        in_offset=bass.IndirectOffsetOnAxis(
                            ap=_fap(idx, [[1, 1]], slot), axis=0),
                    )

                # ---- fractional weights ----
                wxy = pool.tile([P, SLOTS, 2], f32)
                V.tensor_sub(wxy[:], xy[:], fxy[:])
                bs8 = pool.tile([P, SLOTS], f32)
                V.tensor_scalar(bs8[:], mhat[:], 8.0, None, Alu.mult)

                # ---- p4 [P, SLOTS, 2axis, 2which]: (x0, x1, y0, y1) ----
                p4 = pool.tile([P, SLOTS, 2, 2], f32)
                V.tensor_copy(_fap(p4, [[4, SLOTS], [2, 2], [1, 1]]), fxy[:])
                V.tensor_scalar(
                    _fap(p4, [[4, SLOTS], [2, 2], [1, 1]], 1),
                    fxy[:], 1.0, float(W - 1), Alu.add, Alu.min)

                # ---- raw tap positions T = p4 + (j-3)  [P, SLOTS, 4, 7] ----
                t_t = pool.tile([P, SLOTS, 4, KS], f32)
                p4_b = _fap(p4, [[4, SLOTS], [1, 4], [0, KS]])
                jm3 = _fap(m, [[0, SLOTS], [0, 4], [1, KS]], O_JM3)
                V.tensor_add(t_t[:], p4_b, jm3)

                # ---- reflect: R = min(max(T, -T), 2*(W-1) - T) ----
                neg = pool.tile([P, SLOTS, 4, KS], f32)
                a_t = pool.tile([P, SLOTS, 4, KS], f32)
                V.tensor_scalar(neg[:], t_t[:], -1.0, None, Alu.mult)
                V.tensor_tensor(a_t[:], t_t[:], neg[:], op=Alu.max)
                V.tensor_scalar(
                    neg[:], t_t[:], -1.0, float(2 * (W - 1)), Alu.mult, Alu.add)
                r_t = pool.tile([P, SLOTS, 4, KS], f32)
                V.tensor_tensor(r_t[:], a_t[:], neg[:], op=Alu.min)

                # ---- window-relative taps ----
                z_x = pool.tile([P, SLOTS, 2, KS], f32)
                V.tensor_tensor(
                    z_x[:], _fap(r_t, [[4 * KS, SLOTS], [1, 2 * KS]]),
                    _fap(sxc, [[1, SLOTS], [0, 2 * KS]]), op=Alu.subtract)
                z_y = pool.tile([P, SLOTS, 2, KS], f32)
                V.tensor_tensor(
                    z_y[:], _fap(r_t, [[4 * KS, SLOTS], [1, 2 * KS]], 2 * KS),
                    _fap(bs8, [[1, SLOTS], [0, 2 * KS]]), op=Alu.subtract)

                # ---- bilinear weights ww [P, SLOTS, 2axis, 2which] (Pool) ----
                ww = pool.tile([P, SLOTS, 2, 2], f32)
                G.tensor_copy(_fap(ww, [[4, SLOTS], [2, 2], [1, 1]], 1), wxy[:])
                G.tensor_scalar(
                    _fap(ww, [[4, SLOTS], [2, 2], [1, 1]]),
                    wxy[:], -1.0, 1.0, Alu.mult, Alu.add)

                # ---- kwal [P, SLOTS, 4, 7] = ww * k[j] (Pool) ----
                kwal = pool.tile([P, SLOTS, 4, KS], f32)
                ww_b = _fap(ww, [[4, SLOTS], [1, 4], [0, KS]])
                kk_b = _fap(m, [[0, SLOTS], [0, 4], [1, KS]], O_KK)
                G.tensor_tensor(kwal[:], ww_b, kk_b, op=Alu.mult)

                # ---- column weights hw8 [P, SLOTS, 8] (Pool) ----
                eq_c = pool.tile([P, SLOTS, 8, 2 * KS], f32)
                V.tensor_tensor(
                    eq_c[:],
                    _fap(z_x, [[2 * KS, SLOTS], [0, 8], [1, 2 * KS]]),
                    _fap(m, [[0, SLOTS], [1, 8], [0, 2 * KS]], O_IOTA),
                    op=Alu.is_equal)
                V.tensor_tensor(
                    eq_c[:], eq_c[:],
                    _fap(kwal, [[4 * KS, SLOTS], [0, 8], [1, 2 * KS]]),
                    op=Alu.mult)
                hw8 = pool.tile([P, SLOTS, 8], f32)
                V.tensor_reduce(
                    out=_fap(hw8, [[1, SLOTS * 8]]), in_=eq_c[:],
                    axis=Ax.X, op=Alu.add)

                # ---- row weights vw16 [P, SLOTS, 16] (DVE) ----
                eq_r = pool.tile([P, SLOTS, BH, 2 * KS], f32)
                V.tensor_tensor(
                    eq_r[:],
                    _fap(z_y, [[2 * KS, SLOTS], [0, BH], [1, 2 * KS]]),
                    _fap(m, [[0, SLOTS], [1, BH], [0, 2 * KS]], O_IOTA),
                    op=Alu.is_equal)
                V.tensor_tensor(
                    eq_r[:], eq_r[:],
                    _fap(kwal, [[4 * KS, SLOTS], [0, BH], [1, 2 * KS]], 2 * KS),
                    op=Alu.mult)
                vw16 = pool.tile([P, SLOTS, BH], f32)
                V.tensor_reduce(
                    out=_fap(vw16, [[1, SLOTS * BH]]), in_=eq_r[:],
                    axis=Ax.X, op=Alu.add)
                vwc = pool.tile([P, SLOTS, BH, NCH], f32)
                V.tensor_copy(
                    _fap(vwc, [[BH * NCH, SLOTS], [NCH, BH], [1, NCH]]),
                    _fap(vw16, [[BH, SLOTS], [1, BH], [0, NCH]]))

                # ---- apply: tgt[s,c] = sum_{dx,r} hw8[dx] vw16[r] P[dx,r,c] --
                tm1 = pool.tile([P, SLOTS, 8, BH * NCH], f32)
                V.tensor_tensor(
                    tm1[:],
                    _fap(patches, [[RUN, SLOTS], [XSTRIDE, 8], [1, BH * NCH]]),
                    _fap(vwc, [[BH * NCH, SLOTS], [0, 8], [1, BH * NCH]]),
                    op=Alu.mult)
                tm2 = pool.tile([P, SLOTS, 8, NCH], f32)
                V.tensor_reduce(
                    out=_fap(tm2, [[1, SLOTS * 8 * NCH]]),
                    in_=_fap(tm1, [[XSTRIDE, SLOTS * 8], [1, NCH], [NCH, BH]]),
                    axis=Ax.X, op=Alu.add)
                tm3 = pool.tile([P, SLOTS, 8, NCH], f32)
                V.tensor_tensor(
                    tm3[:], tm2[:],
                    _fap(hw8, [[8, SLOTS], [1, 8], [0, NCH]]),
                    op=Alu.mult)
                tgt = pool.tile([P, SLOTS, NCH], f32)
                V.tensor_reduce(
                    out=_fap(tgt, [[1, SLOTS * NCH]]),
                    in_=_fap(tm3, [[8 * NCH, SLOTS], [1, NCH], [NCH, 8]]),
                    axis=Ax.X, op=Alu.add)

                # ---- MSE partial ----
                diff = pool.tile([P, SLOTS, NCH], f32)
                col_ap = _fap(m, [[NCH, SLOTS], [1, NCH]], O_COL)
                V.tensor_tensor(diff[:], tgt[:], col_ap, op=Alu.subtract)
                V.tensor_tensor(diff[:], diff[:], diff[:], op=Alu.mult)
                part = pool.tile([P, 1], f32)
                V.tensor_reduce(
                    out=part[:], in_=_fap(diff, [[1, SLOTS * NCH]]),
                    axis=Ax.X, op=Alu.add)

                nc.sync.dma_start(out=partial[:], in_=part[:])

    split_multi_waits(nc)
    return nc


def make_bands(img_shard):
    """[BPC, 3, 512, 512] -> band layout [b, m, x, r, c] flattened."""
    nhwc = np.ascontiguousarray(
        img_shard.astype(np.float32).transpose(0, 2, 3, 1))
    s0, sH, sW, sC = nhwc.strides
    v = as_strided(nhwc, shape=(BPC, NB, BH, W, NCH),
                   strides=(s0, 8 * sH, sH, sW, sC))
    return np.ascontiguousarray(v.transpose(0, 1, 3, 2, 4)).reshape(-1, 1)


def make_meta(pred_shard):
    """Per-core [P, META_W] meta from the [BPC, L, 8] predictions shard.
    Sample i = slot*P + p."""
    flat = np.ascontiguousarray(pred_shard.reshape(NS, 8).astype(np.float32))
    meta = np.zeros((P, META_W), dtype=np.float32)
    pos = flat[:, :2].reshape(SLOTS, P, 2).transpose(1, 0, 2)
    col = flat[:, 5:8].reshape(SLOTS, P, 3).transpose(1, 0, 2)
    meta[:, O_POS:O_POS + 4] = pos.reshape(P, 4)
    meta[:, O_COL:O_COL + 6] = col.reshape(P, 6)
    meta[:, O_JM3:O_JM3 + KS] = (np.arange(KS, dtype=np.float32) - 3.0)[None, :]
    meta[:, O_KK:O_KK + KS] = _gauss_kernel_np()[None, :]
    p_idx = np.arange(P)
    for slot in range(SLOTS):
        img_i = (slot * P + p_idx) // L
        meta[:, O_IB + slot] = (img_i * IMG_BANDS).astype(np.float32)
    meta[:, O_IOTA:O_IOTA + BH] = np.arange(BH, dtype=np.float32)[None, :]
    return meta


def make_in_maps(predictions, ref_imgs):
    in_maps = []
    for k in range(NCORES):
        in_maps.append({
            "bands": make_bands(ref_imgs[k * BPC:(k + 1) * BPC]),
            "meta": make_meta(predictions[k * BPC:(k + 1) * BPC]),
        })
    return in_maps


_NC_CACHE = {}


def get_nc():
    if "nc" not in _NC_CACHE:
        _NC_CACHE["nc"] = build_bass()
    return _NC_CACHE["nc"]


def _reduce_results(res):
    total = np.float64(0.0)
    for r in res.results:
        total += np.float64(r["partial"].sum(dtype=np.float64))
    return np.float32(total / (B * L * NCH))


def kernel(predictions, ref_imgs):
    predictions = np.asarray(predictions)
    ref_imgs = np.asarray(ref_imgs)
    nc = get_nc()
    in_maps = make_in_maps(predictions, ref_imgs)
    res = run_bass_kernel_spmd(nc, in_maps, list(range(NCORES)))
    return _reduce_results(res)


def run_profiled(predictions, ref_imgs):
    """Like kernel(), but traces with neuron-profile; returns (loss, results)."""
    predictions = np.asarray(predictions)
    ref_imgs = np.asarray(ref_imgs)
    nc = get_nc()
    in_maps = make_in_maps(predictions, ref_imgs)
    res = run_bass_kernel_spmd(
        nc, in_maps, list(range(NCORES)), trace=True)
    return _reduce_results(res), res
